# revision 44
# baseline (speedup 1.0000x reference)
"""LocalSelfAttention (block-diagonal, block=50) Bass kernel for 8 trn2 cores.

Sharding: sequence-parallel with a head-split shared block. Each core owns
5 of the 40 full blocks; the 41st block (tokens 2000-2049) is split by
heads across all 8 cores (2 heads each), so no core computes dead padding
blocks. A host-side permutation of the weight tensors maps each core's 2
shared-block heads to program-heads 0-1, keeping the program SPMD-uniform;
the host sums the 8 partial outputs for the shared block. No collectives.

All matmul data is fp16 (rel err ~8e-4 vs fp32 reference; PSUM accumulation
is fp32; fp8 was measured and fails the 2e-2 gate at ~5e-2). Softmax runs
without max-subtraction (logits are O(+-6) here). Both the pairwise mask
and the attention bias are folded into one per-block additive bias on the
host, injected into the S PSUM bank by a single identity matmul per
block-pair (rows pre-placed at partition bases 0/64).

Performance notes (trn2), each measured on HW:
- Cost of a matmul ~ output FREE columns streamed; all four projections
  keep tokens packed (300/250 cols) in the free dim. The output projection
  is feature-major (yT = Wo^T @ OT, y stored transposed, fp16) - 22% fewer
  columns than token-major. Feature-major V is a LOSS (per-head weight
  DMAs gated by the qkp ring + 82 tiny transposes of PE overhead).
- Wq/Wk live in HBM head-major: one [128, 2048] DMA (4KB lines) delivers a
  full head, so head el's matmuls start when its own 0.5MB lands - no
  all-chunks barrier; x is consumed through a strided packed view of the
  single padded xT tile.
- DMA DIRECT2D triggers execute on the issuing engine's sequencer IN
  PROGRAM ORDER: a trigger placed after an activation waits for it
  (head-of-line blocking). Weights ride sync+gpsimd (no compute there);
  x/bias ride scalar ahead of the exps; finer xT splits measured slower.
- Matmuls whose lhsT partition bases differ (row groups 0 vs 64) can run
  concurrently inside the PE array and must not target the same PSUM
  bank (hardware collision) - AV outputs are split by block parity.
- The kernel is DMA-feed-bound for the first ~100us (34MB of weights at
  ~350GB/s/core) and PE-bound after; ~13us head / ~14us tail are fixed
  sequencer/profiling overheads.
"""

import sys
from contextlib import ExitStack

sys.path.insert(0, "/opt/trn_rl_repo")

import numpy as np

import concourse.bass as bass  # noqa: F401
import concourse.mybir as mybir
import concourse.tile as tile
from concourse import bacc
from concourse.bass_utils import run_bass_kernel_spmd

# ---- problem constants (hardcoded; kernel.py must be self-contained) ----
T, H = 2048, 2048
HEADS, DH = 16, 128
KBLK = 50          # attention block size (tokens)
NEG = -1e9
NCORES = 8
P = T + (KBLK - T % KBLK)          # padded seq len = 2050
NB = P // KBLK                     # 41 real blocks
NB_CORE = 6                        # program blocks per core: 5 own + shared #40
NB_OWN = 5                         # full blocks owned by each core (8*5 = 40)
BPAD = 64                          # per-block padded rows (64-aligned matmul bases)
TPAD = NB_CORE * BPAD              # 384 padded tokens per core
SCALE = DH ** -0.5
NH_T = H // 128                    # 16 h-tiles of 128
TPACK = NB_CORE * KBLK             # 300 packed tokens (program heads 0-1)
TFULL = NB_OWN * KBLK              # 250 packed tokens (program heads 2-15)
F16 = mybir.dt.float16
F32 = mybir.dt.float32

# program block 5 = the shared real block 40; each core computes it only for
# program heads 0-1 (mapped to real heads 2c, 2c+1 by host-side permutation
# of the weight tensors), and the host sums the 8 partial outputs


def _tl(e):
    # packed token-column count for program head e
    return TPACK if e < 2 else TFULL

_CACHE = {}


def _build_program():
    nc = bacc.Bacc("TRN2", target_bir_lowering=False, debug=False,
                   num_devices=NCORES)

    # x laid out h-chunk-major: col h*TPAD + t holds x[t, 128h+p] (fp16)
    xT_d = nc.dram_tensor("xT", [128, NH_T * TPAD], F16, kind="ExternalInput").ap()
    # wq/wk laid out head-major: col e*H + 128*h + m holds W[128h+p, 128e+m],
    # so one [128, 2048] DMA (4KB lines) delivers a full head's weights
    wq_d = nc.dram_tensor("wq", [128, HEADS * H], F16, kind="ExternalInput").ap()
    wk_d = nc.dram_tensor("wk", [128, HEADS * H], F16, kind="ExternalInput").ap()
    wv_d = nc.dram_tensor("wv", [H, H], F16, kind="ExternalInput").ap()
    wo_d = nc.dram_tensor("wo", [H, H], F16, kind="ExternalInput").ap()
    # rows 0-49 / 64-113: bias for even/odd block of a pair; cols (bp, g, el, k)
    bias_d = nc.dram_tensor("bias", [128, 3 * HEADS * KBLK], F16,
                            kind="ExternalInput").ap()
    ident_d = nc.dram_tensor("ident", [128, 128], F16, kind="ExternalInput").ap()
    y_d = nc.dram_tensor("y", [H, TPACK], F16, kind="ExternalOutput").ap()

    with tile.TileContext(nc) as tc, ExitStack() as ctx:
        _emit_body(nc, tc, ctx, xT_d, wq_d, wk_d, wv_d, wo_d, bias_d,
                   ident_d, y_d)

    nc.compile()
    return nc


def _emit_body(nc, tc, ctx, xT_d, wq_d, wk_d, wv_d, wo_d, bias_d,
               ident_d, y_d):
    sb = ctx.enter_context(tc.tile_pool(name="persist", bufs=1))
    qkp = ctx.enter_context(tc.tile_pool(name="qkp", bufs=16))
    wpool = ctx.enter_context(tc.tile_pool(name="wpool", bufs=32))
    apool = ctx.enter_context(tc.tile_pool(name="apool", bufs=3))
    anpool = ctx.enter_context(tc.tile_pool(name="anpool", bufs=7))
    ps_proj = ctx.enter_context(tc.tile_pool(name="ps_proj", bufs=2, space="PSUM"))
    ps_s = ctx.enter_context(tc.tile_pool(name="ps_s", bufs=2, space="PSUM"))
    ps_at = ctx.enter_context(tc.tile_pool(name="ps_at", bufs=2, space="PSUM"))
    ps_ot = ctx.enter_context(tc.tile_pool(name="ps_ot", bufs=2, space="PSUM"))

    # ---- persistent SBUF arrays ----
    xT = sb.tile([128, NH_T * TPAD], F16, tag="xT", name="xT")
    qt = [sb.tile([128, TPACK], F16, tag=f"qt{e}", name=f"qt{e}") for e in range(HEADS)]
    kt = [sb.tile([128, TPACK], F16, tag=f"kt{e}", name=f"kt{e}") for e in range(HEADS)]
    ot = [sb.tile([128, TPACK], F16, tag=f"ot{e}", name=f"ot{e}") for e in range(HEADS)]
    vsb = [sb.tile([128, H], F16, tag=f"v{t}", name=f"vsb{t}") for t in range(3)]
    atb = [sb.tile([128, HEADS * KBLK], F16, tag=f"at{b}", name=f"atb{b}")
           for b in range(NB_CORE)]
    bias_sb = sb.tile([128, 3 * HEADS * KBLK], F16, tag="bias")
    ident = sb.tile([128, 128], F16, tag="ident")

    def wload(dst, src, h):
        # weights alternate the sync/gpsimd queues: BOTH engines carry no
        # compute, so the DIRECT2D triggers never head-of-line block behind
        # activations (the scalar queue stalls on exp; vector on DVE chains)
        (nc.sync if h % 2 == 0 else nc.gpsimd).dma_start(dst, src)

    anrm_live = {}

    def attention_softmax(g):
        # S matmuls + softmax for heads 8g..8g+8 of every block -> anrm tiles.
        # Blocks are processed in pairs sharing [128, .] tiles at partition
        # bases 0/64, so transposes/reductions batch 2 blocks at once.
        # The additive bias for BOTH blocks of the pair is injected by one
        # identity-matmul accumulation into the [128, 400] S PSUM bank
        # (start=True; bias rows pre-placed at partition bases 0/64 on the
        # host) BEFORE the S matmuls (start=False) - all at row group 0, so no
        # concurrent-row-group bank hazard. The softmax scale rides on exp's
        # free affine; bias was pre-scaled by sqrt(DH) on the host.
        for bp in range(NB_CORE // 2):
            asb = apool.tile([128, 8 * BPAD], F16, tag="a_exp", name="asb")
            anrm = anpool.tile([128, 8 * BPAD], F16, tag="a_nrm", name="anrm")
            nc.vector.memset(asb[:], 0.0)   # gap rows feed the batched reduce
            nc.vector.memset(anrm[:], 0.0)  # transpose reads [114, .] spans
            anrm_live[(bp, g)] = anrm
            sums = apool.tile([128, 8], F32, tag="sums", name="sums")
            recip = apool.tile([128, 8], F32, tag="recip", name="recip")
            sp = ps_s.tile([128, 8 * KBLK], F32, tag="s", name="sp")
            boff = (bp * 2 + g) * 8 * KBLK
            nc.tensor.matmul(sp[:], ident[:], bias_sb[:, boff:boff + 8 * KBLK],
                             start=True, stop=False)
            # program block 5 (shared real block 40) exists only for program
            # heads 0-1; its other (par, el) slots hold zero bias (exp -> 1,
            # benign, never read downstream)
            units = [(par, el) for par in range(2) for el in range(8)
                     if not (2 * bp + par == 5 and not (g == 0 and el < 2))]
            for i, (par, el) in enumerate(units):
                b = 2 * bp + par
                pb = BPAD * par
                tcol = KBLK * b
                e = 8 * g + el
                nc.tensor.matmul(sp[pb:pb + KBLK, KBLK * el:KBLK * (el + 1)],
                                 qt[e][:, tcol:tcol + KBLK],
                                 kt[e][:, tcol:tcol + KBLK],
                                 start=False, stop=(i == len(units) - 1))
            nc.scalar.activation(
                asb[0:2 * BPAD - 14, :].rearrange(
                    "p (e x) -> p e x", e=8)[:, :, 0:KBLK],
                sp[0:2 * BPAD - 14, :].rearrange("p (e x) -> p e x", e=8),
                mybir.ActivationFunctionType.Exp, scale=SCALE)
            nc.vector.reduce_sum(
                sums[:], asb.rearrange("p (e x) -> p e x", e=8)[:, :, 0:KBLK],
                axis=mybir.AxisListType.X)
            nc.vector.reciprocal(recip[:], sums[:])
            for par, el in units:
                pb = BPAD * par
                nc.vector.tensor_scalar_mul(
                    anrm[pb:pb + KBLK, BPAD * el:BPAD * el + KBLK],
                    asb[pb:pb + KBLK, BPAD * el:BPAD * el + KBLK],
                    recip[pb:pb + KBLK, el:el + 1])

    def attention_transpose(g):
        # emitted a phase later so the PE never stalls on the softmax chain;
        # one [114, 114] transpose covers 2 blocks x 2 heads
        for bp in range(NB_CORE // 2):
            anrm = anrm_live.pop((bp, g))
            for j in range(4):
                atp = ps_at.tile([128, 2 * BPAD - 14], F16, tag="atp", name="atp")
                nc.tensor.transpose(
                    atp[0:2 * BPAD - 14, :],
                    anrm[0:2 * BPAD - 14, 2 * BPAD * j:2 * BPAD * (j + 1) - 14],
                    ident[0:2 * BPAD - 14, 0:2 * BPAD - 14])
                for p_ in range(2):
                    e = 8 * g + 2 * j + p_
                    for par in range(2):
                        b = 2 * bp + par
                        if b == 5 and e >= 2:
                            continue
                        base = BPAD * par
                        nc.vector.tensor_copy(
                            atb[b][base:base + KBLK, KBLK * e:KBLK * (e + 1)],
                            atp[BPAD * p_:BPAD * p_ + KBLK,
                                BPAD * par:BPAD * par + KBLK])

    # ---- x / bias / ident on the scalar DMA queue: these triggers sit at
    # the head of the scalar stream (long before the exps that later block
    # it), so they issue immediately; xT is split so its first part lands
    # before the first QT matmuls need it. (Measured slower alternatives:
    # 8-way split; spreading parts onto the sync/gpsimd weight queues.)
    for part in range(4):
        w = NH_T * TPAD // 4
        nc.scalar.dma_start(xT[:, w * part:w * (part + 1)],
                            xT_d[:, w * part:w * (part + 1)])
    nc.scalar.dma_start(bias_sb[:], bias_d)
    nc.scalar.dma_start(ident[:], ident_d)

    def xq(h):
        # packed-300-token view of x chunk h (drops the 14 pad rows per block)
        return xT[:, TPAD * h:TPAD * (h + 1)].rearrange(
            "p (b x) -> p b x", b=NB_CORE)[:, :, 0:KBLK]

    # ---- QT/KT projections (per half of the heads) + interleaved attention --
    # Weights arrive one full head per DMA, so head el's matmuls start as
    # soon as its own 0.5MB lands - no all-chunks barrier at phase start.
    # (Emitting all four load sets up front measured ~1.5us slower: the qkp
    # ring's eviction semaphores already pace the transfers.)
    for g in range(2):
        for w_hbm, dst in ((wq_d, qt), (wk_d, kt)):
            wt = [qkp.tile([128, H], F16, tag="wqk", name="wqk")
                  for _ in range(8)]
            for el in range(8):
                wload(wt[el][:], w_hbm[:, H * (8 * g + el):H * (8 * g + el + 1)],
                      el)
            if g == 1 and w_hbm is wk_d:
                # prefetch the V group-0 chunks on the SCALAR queue: its
                # triggers sit behind the exp-g0 activations and fire at
                # ~48us - after the early weight crunch, well before V needs
                # them - and the weight queues stream wk-g1/wo uncontended
                # (interleaving wv INTO the wk-g1 stream measured slower)
                wv_pref = [wpool.tile([128, 1024], F16, tag="w", name="w")
                           for _ in range(NH_T)]
                for h in range(NH_T):
                    nc.scalar.dma_start(wv_pref[h][:],
                                        wv_d[128 * h:128 * (h + 1), 0:1024])
            for el in range(8):            # head within group
                e = 8 * g + el
                tl = _tl(e)
                nb = tl // KBLK
                pt = ps_proj.tile([128, TPACK], F32, tag="proj", name="pt")
                for h in range(NH_T):
                    nc.tensor.matmul(pt[:, 0:tl],
                                     wt[el][:, 128 * h:128 * (h + 1)],
                                     xq(h)[:, 0:nb, :],
                                     start=(h == 0), stop=(h == NH_T - 1))
                nc.vector.tensor_copy(dst[e][:, 0:tl], pt[:, 0:tl])
        attention_softmax(g)
        if g == 1:
            attention_transpose(0)

    # ---- V projection, token-major: out[t, ed] = xT[h, t].T @ W[h, ed] ----
    # (feature-major V measured slower: its per-head weight DMAs are gated
    # by the qkp ring so V stalls ~7us, and 82 tiny block-transposes cost
    # ~9us of PE instruction overhead)
    wt_wo0 = None
    for g in range(2):
        if g == 0:
            wt = wv_pref
        else:
            wt = [wpool.tile([128, 1024], F16, tag="w", name="w")
                  for _ in range(NH_T)]
            for h in range(NH_T):
                wload(wt[h][:], wv_d[128 * h:128 * (h + 1), 1024:2048], h)
        if g == 1:
            # stage Wo group 0 now; its slot-waits resolve as V-g0 chunks
            # release, well before the output projection needs the data
            wt_wo0 = [wpool.tile([128, 1024], F16, tag="w", name="w")
                      for _ in range(HEADS)]
            for e in range(HEADS):
                wload(wt_wo0[e][:], wo_d[128 * e:128 * (e + 1), 0:1024], e + 1)
        for eo in range(2):
            eg = 2 * g + eo
            for tt in range(3):            # token tiles of 128
                pt = ps_proj.tile([128, 512], F32, tag="proj", name="pt")
                for h in range(NH_T):
                    nc.tensor.matmul(pt[:], xT[:, TPAD * h + 128 * tt:
                                               TPAD * h + 128 * (tt + 1)],
                                     wt[h][:, 512 * eo:512 * (eo + 1)],
                                     start=(h == 0), stop=(h == NH_T - 1))
                nc.scalar.copy(vsb[tt][:, 512 * eg:512 * (eg + 1)], pt[:])

    attention_transpose(1)

    # stage Wo group 1 now: the V matmuls holding wpool slots are already
    # done, so the DMAs run during the AV phase, well before hg=8 needs them
    wt_wo1 = [wpool.tile([128, 1024], F16, tag="w", name="w")
              for _ in range(HEADS)]
    for e in range(HEADS):
        wload(wt_wo1[e][:], wo_d[128 * e:128 * (e + 1), 1024:2048], e)

    # ---- A^T @ V -> OT[dh, t] per head (ot packed to 300 token columns) ----
    # Matmuls with different lhsT partition bases (row groups 0 vs 64) run
    # concurrently in the PE array and must not share a PSUM bank: use one
    # PSUM tile per block-parity, then strided copies into ot[e].
    for e in range(HEADS):
        nblk = NB_CORE if e < 2 else NB_OWN
        opa = ps_ot.tile([128, TPAD], F32, tag="otp", name="opa")
        opb = ps_ot.tile([128, TPAD], F32, tag="otp", name="opb")
        opp = (opa, opb)
        for b in range(nblk):
            par = b % 2
            base = BPAD * par
            nc.tensor.matmul(
                opp[par][:, 128 * (b // 2):128 * (b // 2) + KBLK],
                vsb[b // 2][base:base + KBLK, 128 * e:128 * (e + 1)],
                atb[b][base:base + KBLK, KBLK * e:KBLK * (e + 1)],
                start=True, stop=True)
        for par in range(2):
            nb = (nblk - par + 1) // 2     # block-halves written for this par
            src = opp[par].rearrange("p (b x) -> p b x", b=3)[:, 0:nb, 0:KBLK]
            dst = ot[e].rearrange("p (b x) -> p b x", b=3)[
                :, 0:nb, KBLK * par:KBLK * (par + 1)]
            nc.scalar.copy(dst, src)

    # ---- output projection, feature-major: yT[hout, t] = Wo^T @ OT ----
    # Streaming the packed 300 token columns (instead of 512 hout columns
    # over padded token tiles) cuts the PE column count by 22%; y is stored
    # transposed [H, 300] fp16 and unscrambled on the host.
    for hg in range(HEADS):
        wt = wt_wo0 if hg < 8 else wt_wo1
        ho = hg % 8
        pt = ps_proj.tile([128, TPACK], F32, tag="proj", name="pt")
        for e in range(HEADS):
            tl = _tl(e)
            nc.tensor.matmul(pt[:, 0:tl], wt[e][:, 128 * ho:128 * (ho + 1)],
                             ot[e][:, 0:tl], start=(e == 0),
                             stop=(e == HEADS - 1))
        ysb = apool.tile([128, TPACK], F16, tag="y", name="ysb")
        if hg % 2 == 0:
            nc.vector.tensor_copy(ysb[:], pt[:])
            nc.sync.dma_start(y_d[128 * hg:128 * (hg + 1), :], ysb[:])
        else:
            nc.scalar.copy(ysb[:], pt[:])
            nc.gpsimd.dma_start(y_d[128 * hg:128 * (hg + 1), :], ysb[:])


def _prep_inputs(x, mask, bias, Wq, Wk, Wv, Wo):
    """Slice/pad/transpose the full inputs into per-core input maps."""
    x = np.asarray(x, np.float32).reshape(T, H)
    mask = np.asarray(mask, np.float32).reshape(T)
    bias = np.asarray(bias, np.float32).reshape(HEADS, T, T)

    # padded x (tokens) and mask, as in the reference
    xp = np.zeros((P, H), np.float32)
    xp[:T] = x
    mb = np.full(P, NEG, np.float32)
    mb[:T] = mask

    # combined per-block additive bias: block-diag of attention bias + pair mask
    comb = np.zeros((NB_CORE * NCORES, HEADS, KBLK, KBLK), np.float32)
    for b in range(NB):
        lo, hi = KBLK * b, KBLK * (b + 1)
        blk = np.zeros((HEADS, KBLK, KBLK), np.float32)
        lim = min(hi, T) - lo
        if lim > 0:
            blk[:, :lim, :lim] = bias[:, lo:lo + lim, lo:lo + lim]
        pair = mb[lo:hi, None] * mb[None, lo:hi]
        # pre-scaled by sqrt(DH) (exp applies scale=DH**-0.5 to S + bias);
        # masked entries use -4000 (fp16-safe; * SCALE -> -354, exp -> 0)
        blk = blk * (DH ** 0.5) + np.where(pair > 0, 0.0, -4000.0)[None]
        comb[b] = blk

    def headmajor(w):
        # [128, e*H + 128*h + m] <- W[128h+p, 128e+m]
        a = np.asarray(w, np.float32).reshape(NH_T, 128, HEADS, DH)
        return np.ascontiguousarray(
            a.transpose(1, 2, 0, 3).reshape(128, HEADS, H))

    wqh = headmajor(Wq)
    wkh = headmajor(Wk)
    wv3 = np.asarray(Wv, np.float32).reshape(H, HEADS, DH)
    wo3 = np.asarray(Wo, np.float32).reshape(HEADS, DH, H)
    ident = np.eye(128, dtype=np.float16)

    in_maps = []
    for c in range(NCORES):
        # program-head order: real heads 2c, 2c+1 first (they own the shared
        # block 40 on this core), then the rest in order
        perm = [2 * c, 2 * c + 1] + [e for e in range(HEADS)
                                     if e not in (2 * c, 2 * c + 1)]
        xc = np.zeros((TPAD, H), np.float32)
        for j in range(NB_CORE):
            b = NB_OWN * c + j if j < NB_OWN else NB - 1
            xc[BPAD * j:BPAD * j + KBLK] = xp[KBLK * b:KBLK * (b + 1)]
        # bias layout: rows 0-49 / 64-113 = even/odd block of each pair,
        # cols (blockpair, headgroup, head, k); injected by one ident matmul.
        # Block 5 (= real block 40) only has program heads 0-1; its other
        # slots stay zero.
        bc = np.zeros((128, 3 * HEADS * KBLK), np.float16)
        for b in range(NB_CORE):
            if b < NB_OWN:
                blkbias = comb[NB_OWN * c + b][perm]      # [16, 50, 50]
            else:
                blkbias = np.zeros((HEADS, KBLK, KBLK), np.float32)
                blkbias[0:2] = comb[NB - 1][[2 * c, 2 * c + 1]]
            bp, par = b // 2, b % 2
            for g in range(2):
                blk = blkbias[8 * g:8 * (g + 1)]          # [8, 50, 50]
                bc[BPAD * par:BPAD * par + KBLK,
                   (bp * 2 + g) * 8 * KBLK:(bp * 2 + g + 1) * 8 * KBLK] = (
                    blk.transpose(1, 0, 2).reshape(KBLK, 8 * KBLK))
        # xT layout: [128, h*TPAD + t] <- x[t, 128h+p]
        xh = np.ascontiguousarray(
            xc.T.reshape(NH_T, 128, TPAD).transpose(1, 0, 2).reshape(
                128, NH_T * TPAD)).astype(np.float16)
        in_maps.append({
            "xT": xh,
            "wq": np.ascontiguousarray(
                wqh[:, perm].reshape(128, HEADS * H)).astype(np.float16),
            "wk": np.ascontiguousarray(
                wkh[:, perm].reshape(128, HEADS * H)).astype(np.float16),
            "wv": np.ascontiguousarray(
                wv3[:, perm].reshape(H, H)).astype(np.float16),
            "wo": np.ascontiguousarray(
                wo3[perm].reshape(H, H)).astype(np.float16),
            "bias": np.ascontiguousarray(bc),
            "ident": ident,
        })
    return in_maps


def _gather(results):
    out = np.empty((T, H), np.float32)
    y40 = np.zeros((KBLK, H), np.float32)
    for c in range(NCORES):
        yc = np.asarray(results[c]["y"], np.float32).T   # [TPACK, H]
        for j in range(NB_OWN):
            b = NB_OWN * c + j
            out[KBLK * b:KBLK * (b + 1)] = yc[KBLK * j:KBLK * (j + 1)]
        # cols 250-300 hold this core's 2-head partial of shared block 40
        y40 += yc[TFULL:TPACK]
    n40 = T - KBLK * (NB - 1)                            # 48 real rows
    out[KBLK * (NB - 1):] = y40[:n40]
    return out


def run(trace=False, **inputs):
    if "nc" not in _CACHE:
        _CACHE["nc"] = _build_program()
    nc = _CACHE["nc"]
    in_maps = _prep_inputs(
        inputs["x_BxTxH"], inputs["mask_BxT"], inputs["attention_bias_BxHxTxT"],
        inputs["Wq"], inputs["Wk"], inputs["Wv"], inputs["Wo"])
    res = run_bass_kernel_spmd(nc, in_maps, list(range(NCORES)), trace=trace)
    out = _gather(res.results)[None]       # restore batch dim [1, T, H]
    return out, res.exec_time_ns


def kernel(**inputs):
    out, _ = run(trace=False, **inputs)
    return out



# revision 45
# speedup vs baseline: 1.0400x; 1.0400x over previous
"""LocalSelfAttention (block-diagonal, block=50) Bass kernel for 8 trn2 cores.

Sharding: sequence-parallel with a head-split shared block. Each core owns
5 of the 40 full blocks; the 41st block (tokens 2000-2049) is split by
heads across all 8 cores (2 heads each), so no core computes dead padding
blocks. A host-side permutation of the weight tensors maps each core's 2
shared-block heads to program-heads 0-1, keeping the program SPMD-uniform;
the host sums the 8 partial outputs for the shared block. No collectives.

All matmul data is fp16 (rel err ~8e-4 vs fp32 reference; PSUM accumulation
is fp32; fp8 was measured and fails the 2e-2 gate at ~5e-2). Softmax runs
without max-subtraction (logits are O(+-6) here). Both the pairwise mask
and the attention bias are folded into one per-block additive bias on the
host, injected into the S PSUM bank by a single identity matmul per
block-pair (rows pre-placed at partition bases 0/64).

Performance notes (trn2), each measured on HW:
- Cost of a matmul ~ output FREE columns streamed; all four projections
  keep tokens packed (300/250 cols) in the free dim. The output projection
  is feature-major (yT = Wo^T @ OT, y stored transposed, fp16) - 22% fewer
  columns than token-major. Feature-major V is a LOSS (per-head weight
  DMAs gated by the qkp ring + 82 tiny transposes of PE overhead).
- Wq/Wk live in HBM head-major: one [128, 2048] DMA (4KB lines) delivers a
  full head, so head el's matmuls start when its own 0.5MB lands - no
  all-chunks barrier; x is consumed through a strided packed view of the
  single padded xT tile.
- DMA DIRECT2D triggers execute on the issuing engine's sequencer IN
  PROGRAM ORDER: a trigger placed after an activation waits for it
  (head-of-line blocking). Weights ride sync+gpsimd (no compute there);
  x/bias ride scalar ahead of the exps; finer xT splits measured slower.
- Matmuls whose lhsT partition bases differ (row groups 0 vs 64) can run
  concurrently inside the PE array and must not target the same PSUM
  bank (hardware collision) - AV outputs are split by block parity.
- The kernel is DMA-feed-bound for the first ~100us (34MB of weights at
  ~350GB/s/core) and PE-bound after; ~13us head / ~14us tail are fixed
  sequencer/profiling overheads.
"""

import sys
from contextlib import ExitStack

sys.path.insert(0, "/opt/trn_rl_repo")

import numpy as np

import concourse.bass as bass  # noqa: F401
import concourse.mybir as mybir
import concourse.tile as tile
from concourse import bacc
from concourse.bass_utils import run_bass_kernel_spmd

# ---- problem constants (hardcoded; kernel.py must be self-contained) ----
T, H = 2048, 2048
HEADS, DH = 16, 128
KBLK = 50          # attention block size (tokens)
NEG = -1e9
NCORES = 8
P = T + (KBLK - T % KBLK)          # padded seq len = 2050
NB = P // KBLK                     # 41 real blocks
NB_CORE = 6                        # program blocks per core: 5 own + shared #40
NB_OWN = 5                         # full blocks owned by each core (8*5 = 40)
BPAD = 64                          # per-block padded rows (64-aligned matmul bases)
TPAD = NB_CORE * BPAD              # 384 padded tokens per core
SCALE = DH ** -0.5
NH_T = H // 128                    # 16 h-tiles of 128
TPACK = NB_CORE * KBLK             # 300 packed tokens (program heads 0-1)
TFULL = NB_OWN * KBLK              # 250 packed tokens (program heads 2-15)
F16 = mybir.dt.float16
F32 = mybir.dt.float32

# program block 5 = the shared real block 40; each core computes it only for
# program heads 0-1 (mapped to real heads 2c, 2c+1 by host-side permutation
# of the weight tensors), and the host sums the 8 partial outputs


def _tl(e):
    # packed token-column count for program head e
    return TPACK if e < 2 else TFULL

_CACHE = {}


def _build_program():
    nc = bacc.Bacc("TRN2", target_bir_lowering=False, debug=False,
                   num_devices=NCORES)

    # x laid out h-chunk-major: col h*TPAD + t holds x[t, 128h+p] (fp16)
    xT_d = nc.dram_tensor("xT", [128, NH_T * TPAD], F16, kind="ExternalInput").ap()
    # wq/wk laid out head-major: col e*H + 128*h + m holds W[128h+p, 128e+m],
    # so one [128, 2048] DMA (4KB lines) delivers a full head's weights
    wq_d = nc.dram_tensor("wq", [128, HEADS * H], F16, kind="ExternalInput").ap()
    wk_d = nc.dram_tensor("wk", [128, HEADS * H], F16, kind="ExternalInput").ap()
    wv_d = nc.dram_tensor("wv", [H, H], F16, kind="ExternalInput").ap()
    wo_d = nc.dram_tensor("wo", [H, H], F16, kind="ExternalInput").ap()
    # rows 0-49 / 64-113: bias for even/odd block of a pair; cols (bp, g, el, k)
    bias_d = nc.dram_tensor("bias", [128, 3 * HEADS * KBLK], F16,
                            kind="ExternalInput").ap()
    ident_d = nc.dram_tensor("ident", [128, 128], F16, kind="ExternalInput").ap()
    y_d = nc.dram_tensor("y", [H, TPACK], F16, kind="ExternalOutput").ap()

    with tile.TileContext(nc) as tc, ExitStack() as ctx:
        _emit_body(nc, tc, ctx, xT_d, wq_d, wk_d, wv_d, wo_d, bias_d,
                   ident_d, y_d)

    nc.compile()
    return nc


def _emit_body(nc, tc, ctx, xT_d, wq_d, wk_d, wv_d, wo_d, bias_d,
               ident_d, y_d):
    sb = ctx.enter_context(tc.tile_pool(name="persist", bufs=1))
    qkp = ctx.enter_context(tc.tile_pool(name="qkp", bufs=16))
    wpool = ctx.enter_context(tc.tile_pool(name="wpool", bufs=32))
    apool = ctx.enter_context(tc.tile_pool(name="apool", bufs=3))
    anpool = ctx.enter_context(tc.tile_pool(name="anpool", bufs=7))
    ps_proj = ctx.enter_context(tc.tile_pool(name="ps_proj", bufs=2, space="PSUM"))
    ps_s = ctx.enter_context(tc.tile_pool(name="ps_s", bufs=2, space="PSUM"))
    ps_at = ctx.enter_context(tc.tile_pool(name="ps_at", bufs=2, space="PSUM"))
    ps_ot = ctx.enter_context(tc.tile_pool(name="ps_ot", bufs=2, space="PSUM"))

    # ---- persistent SBUF arrays ----
    xT = sb.tile([128, NH_T * TPAD], F16, tag="xT", name="xT")
    qt = [sb.tile([128, TPACK], F16, tag=f"qt{e}", name=f"qt{e}") for e in range(HEADS)]
    kt = [sb.tile([128, TPACK], F16, tag=f"kt{e}", name=f"kt{e}") for e in range(HEADS)]
    ot = [sb.tile([128, TPACK], F16, tag=f"ot{e}", name=f"ot{e}") for e in range(HEADS)]
    vsb = [sb.tile([128, H], F16, tag=f"v{t}", name=f"vsb{t}") for t in range(3)]
    atb = [sb.tile([128, HEADS * KBLK], F16, tag=f"at{b}", name=f"atb{b}")
           for b in range(NB_CORE)]
    bias_sb = sb.tile([128, 3 * HEADS * KBLK], F16, tag="bias")
    ident = sb.tile([128, 128], F16, tag="ident")

    def wload(dst, src, h):
        # weights alternate the sync/gpsimd queues: BOTH engines carry no
        # compute, so the DIRECT2D triggers never head-of-line block behind
        # activations (the scalar queue stalls on exp; vector on DVE chains)
        (nc.sync if h % 2 == 0 else nc.gpsimd).dma_start(dst, src)

    anrm_live = {}

    def attention_softmax(g):
        # S matmuls + softmax for heads 8g..8g+8 of every block -> anrm tiles.
        # Blocks are processed in pairs sharing [128, .] tiles at partition
        # bases 0/64, so transposes/reductions batch 2 blocks at once.
        # The additive bias for BOTH blocks of the pair is injected by one
        # identity-matmul accumulation into the [128, 400] S PSUM bank
        # (start=True; bias rows pre-placed at partition bases 0/64 on the
        # host) BEFORE the S matmuls (start=False) - all at row group 0, so no
        # concurrent-row-group bank hazard. The softmax scale rides on exp's
        # free affine; bias was pre-scaled by sqrt(DH) on the host.
        for bp in range(NB_CORE // 2):
            asb = apool.tile([128, 8 * BPAD], F16, tag="a_exp", name="asb")
            anrm = anpool.tile([128, 8 * BPAD], F16, tag="a_nrm", name="anrm")
            nc.vector.memset(asb[:], 0.0)   # gap rows feed the batched reduce
            nc.vector.memset(anrm[:], 0.0)  # transpose reads [114, .] spans
            anrm_live[(bp, g)] = anrm
            sums = apool.tile([128, 8], F32, tag="sums", name="sums")
            recip = apool.tile([128, 8], F32, tag="recip", name="recip")
            sp = ps_s.tile([128, 8 * KBLK], F32, tag="s", name="sp")
            boff = (bp * 2 + g) * 8 * KBLK
            nc.tensor.matmul(sp[:], ident[:], bias_sb[:, boff:boff + 8 * KBLK],
                             start=True, stop=False)
            # program block 5 (shared real block 40) exists only for program
            # heads 0-1; its other (par, el) slots hold zero bias (exp -> 1,
            # benign, never read downstream)
            units = [(par, el) for par in range(2) for el in range(8)
                     if not (2 * bp + par == 5 and not (g == 0 and el < 2))]
            for i, (par, el) in enumerate(units):
                b = 2 * bp + par
                pb = BPAD * par
                tcol = KBLK * b
                e = 8 * g + el
                nc.tensor.matmul(sp[pb:pb + KBLK, KBLK * el:KBLK * (el + 1)],
                                 qt[e][:, tcol:tcol + KBLK],
                                 kt[e][:, tcol:tcol + KBLK],
                                 start=False, stop=(i == len(units) - 1))
            nc.scalar.activation(
                asb[0:2 * BPAD - 14, :].rearrange(
                    "p (e x) -> p e x", e=8)[:, :, 0:KBLK],
                sp[0:2 * BPAD - 14, :].rearrange("p (e x) -> p e x", e=8),
                mybir.ActivationFunctionType.Exp, scale=SCALE)
            nc.vector.reduce_sum(
                sums[:], asb.rearrange("p (e x) -> p e x", e=8)[:, :, 0:KBLK],
                axis=mybir.AxisListType.X)
            nc.vector.reciprocal(recip[:], sums[:])
            for par, el in units:
                pb = BPAD * par
                nc.vector.tensor_scalar_mul(
                    anrm[pb:pb + KBLK, BPAD * el:BPAD * el + KBLK],
                    asb[pb:pb + KBLK, BPAD * el:BPAD * el + KBLK],
                    recip[pb:pb + KBLK, el:el + 1])

    def attention_transpose(g):
        # emitted a phase later so the PE never stalls on the softmax chain;
        # one [114, 114] transpose covers 2 blocks x 2 heads
        for bp in range(NB_CORE // 2):
            anrm = anrm_live.pop((bp, g))
            for j in range(4):
                atp = ps_at.tile([128, 2 * BPAD - 14], F16, tag="atp", name="atp")
                nc.tensor.transpose(
                    atp[0:2 * BPAD - 14, :],
                    anrm[0:2 * BPAD - 14, 2 * BPAD * j:2 * BPAD * (j + 1) - 14],
                    ident[0:2 * BPAD - 14, 0:2 * BPAD - 14])
                for p_ in range(2):
                    e = 8 * g + 2 * j + p_
                    for par in range(2):
                        b = 2 * bp + par
                        if b == 5 and e >= 2:
                            continue
                        base = BPAD * par
                        nc.vector.tensor_copy(
                            atb[b][base:base + KBLK, KBLK * e:KBLK * (e + 1)],
                            atp[BPAD * p_:BPAD * p_ + KBLK,
                                BPAD * par:BPAD * par + KBLK])

    # ---- x / bias / ident on the scalar DMA queue: these triggers sit at
    # the head of the scalar stream (long before the exps that later block
    # it), so they issue immediately; xT is split so its first part lands
    # before the first QT matmuls need it. (Measured slower alternatives:
    # 8-way split; spreading parts onto the sync/gpsimd weight queues.)
    for part in range(4):
        w = NH_T * TPAD // 4
        nc.scalar.dma_start(xT[:, w * part:w * (part + 1)],
                            xT_d[:, w * part:w * (part + 1)])
    nc.scalar.dma_start(bias_sb[:], bias_d)
    nc.scalar.dma_start(ident[:], ident_d)

    def xq(h):
        # packed-300-token view of x chunk h (drops the 14 pad rows per block)
        return xT[:, TPAD * h:TPAD * (h + 1)].rearrange(
            "p (b x) -> p b x", b=NB_CORE)[:, :, 0:KBLK]

    # ---- QT/KT projections (per half of the heads) + interleaved attention --
    # Weights arrive one full head per DMA, so head el's matmuls start as
    # soon as its own 0.5MB lands - no all-chunks barrier at phase start.
    # (Emitting all four load sets up front measured ~1.5us slower: the qkp
    # ring's eviction semaphores already pace the transfers.)
    for g in range(2):
        for w_hbm, dst in ((wq_d, qt), (wk_d, kt)):
            wt = [qkp.tile([128, H], F16, tag="wqk", name="wqk")
                  for _ in range(8)]
            for el in range(8):
                wload(wt[el][:], w_hbm[:, H * (8 * g + el):H * (8 * g + el + 1)],
                      el)
            if g == 1 and w_hbm is wk_d:
                # prefetch the V group-0 chunks: fresh wpool slots, so the
                # DMAs stream right behind wk-g1 instead of at V entry.
                # (Measured slower: interleaving wv INTO the wk-g1 stream;
                # routing all 16 chunks through the scalar queue, +10us -
                # one queue can't stream 4.2MB fast enough.)
                wv_pref = [wpool.tile([128, 1024], F16, tag="w", name="w")
                           for _ in range(NH_T)]
                for h in range(NH_T):
                    wload(wv_pref[h][:], wv_d[128 * h:128 * (h + 1), 0:1024], h)
            for el in range(8):            # head within group
                e = 8 * g + el
                tl = _tl(e)
                nb = tl // KBLK
                pt = ps_proj.tile([128, TPACK], F32, tag="proj", name="pt")
                for h in range(NH_T):
                    nc.tensor.matmul(pt[:, 0:tl],
                                     wt[el][:, 128 * h:128 * (h + 1)],
                                     xq(h)[:, 0:nb, :],
                                     start=(h == 0), stop=(h == NH_T - 1))
                nc.vector.tensor_copy(dst[e][:, 0:tl], pt[:, 0:tl])
        attention_softmax(g)
        if g == 1:
            attention_transpose(0)

    # ---- V projection, token-major: out[t, ed] = xT[h, t].T @ W[h, ed] ----
    # (feature-major V measured slower: its per-head weight DMAs are gated
    # by the qkp ring so V stalls ~7us, and 82 tiny block-transposes cost
    # ~9us of PE instruction overhead)
    wt_wo0 = None
    for g in range(2):
        if g == 0:
            wt = wv_pref
        else:
            wt = [wpool.tile([128, 1024], F16, tag="w", name="w")
                  for _ in range(NH_T)]
            for h in range(NH_T):
                wload(wt[h][:], wv_d[128 * h:128 * (h + 1), 1024:2048], h)
        if g == 1:
            # stage Wo group 0 now; its slot-waits resolve as V-g0 chunks
            # release, well before the output projection needs the data
            wt_wo0 = [wpool.tile([128, 1024], F16, tag="w", name="w")
                      for _ in range(HEADS)]
            for e in range(HEADS):
                wload(wt_wo0[e][:], wo_d[128 * e:128 * (e + 1), 0:1024], e + 1)
        for eo in range(2):
            eg = 2 * g + eo
            for tt in range(3):            # token tiles of 128
                pt = ps_proj.tile([128, 512], F32, tag="proj", name="pt")
                for h in range(NH_T):
                    nc.tensor.matmul(pt[:], xT[:, TPAD * h + 128 * tt:
                                               TPAD * h + 128 * (tt + 1)],
                                     wt[h][:, 512 * eo:512 * (eo + 1)],
                                     start=(h == 0), stop=(h == NH_T - 1))
                nc.scalar.copy(vsb[tt][:, 512 * eg:512 * (eg + 1)], pt[:])

    attention_transpose(1)

    # stage Wo group 1 now: the V matmuls holding wpool slots are already
    # done, so the DMAs run during the AV phase, well before hg=8 needs them
    wt_wo1 = [wpool.tile([128, 1024], F16, tag="w", name="w")
              for _ in range(HEADS)]
    for e in range(HEADS):
        wload(wt_wo1[e][:], wo_d[128 * e:128 * (e + 1), 1024:2048], e)

    # ---- A^T @ V -> OT[dh, t] per head (ot packed to 300 token columns) ----
    # Matmuls with different lhsT partition bases (row groups 0 vs 64) run
    # concurrently in the PE array and must not share a PSUM bank: use one
    # PSUM tile per block-parity, then strided copies into ot[e].
    for e in range(HEADS):
        nblk = NB_CORE if e < 2 else NB_OWN
        opa = ps_ot.tile([128, TPAD], F32, tag="otp", name="opa")
        opb = ps_ot.tile([128, TPAD], F32, tag="otp", name="opb")
        opp = (opa, opb)
        for b in range(nblk):
            par = b % 2
            base = BPAD * par
            nc.tensor.matmul(
                opp[par][:, 128 * (b // 2):128 * (b // 2) + KBLK],
                vsb[b // 2][base:base + KBLK, 128 * e:128 * (e + 1)],
                atb[b][base:base + KBLK, KBLK * e:KBLK * (e + 1)],
                start=True, stop=True)
        for par in range(2):
            nb = (nblk - par + 1) // 2     # block-halves written for this par
            src = opp[par].rearrange("p (b x) -> p b x", b=3)[:, 0:nb, 0:KBLK]
            dst = ot[e].rearrange("p (b x) -> p b x", b=3)[
                :, 0:nb, KBLK * par:KBLK * (par + 1)]
            nc.scalar.copy(dst, src)

    # ---- output projection, feature-major: yT[hout, t] = Wo^T @ OT ----
    # Streaming the packed 300 token columns (instead of 512 hout columns
    # over padded token tiles) cuts the PE column count by 22%; y is stored
    # transposed [H, 300] fp16 and unscrambled on the host.
    for hg in range(HEADS):
        wt = wt_wo0 if hg < 8 else wt_wo1
        ho = hg % 8
        pt = ps_proj.tile([128, TPACK], F32, tag="proj", name="pt")
        for e in range(HEADS):
            tl = _tl(e)
            nc.tensor.matmul(pt[:, 0:tl], wt[e][:, 128 * ho:128 * (ho + 1)],
                             ot[e][:, 0:tl], start=(e == 0),
                             stop=(e == HEADS - 1))
        ysb = apool.tile([128, TPACK], F16, tag="y", name="ysb")
        if hg % 2 == 0:
            nc.vector.tensor_copy(ysb[:], pt[:])
            nc.sync.dma_start(y_d[128 * hg:128 * (hg + 1), :], ysb[:])
        else:
            nc.scalar.copy(ysb[:], pt[:])
            nc.gpsimd.dma_start(y_d[128 * hg:128 * (hg + 1), :], ysb[:])


def _prep_inputs(x, mask, bias, Wq, Wk, Wv, Wo):
    """Slice/pad/transpose the full inputs into per-core input maps."""
    x = np.asarray(x, np.float32).reshape(T, H)
    mask = np.asarray(mask, np.float32).reshape(T)
    bias = np.asarray(bias, np.float32).reshape(HEADS, T, T)

    # padded x (tokens) and mask, as in the reference
    xp = np.zeros((P, H), np.float32)
    xp[:T] = x
    mb = np.full(P, NEG, np.float32)
    mb[:T] = mask

    # combined per-block additive bias: block-diag of attention bias + pair mask
    comb = np.zeros((NB_CORE * NCORES, HEADS, KBLK, KBLK), np.float32)
    for b in range(NB):
        lo, hi = KBLK * b, KBLK * (b + 1)
        blk = np.zeros((HEADS, KBLK, KBLK), np.float32)
        lim = min(hi, T) - lo
        if lim > 0:
            blk[:, :lim, :lim] = bias[:, lo:lo + lim, lo:lo + lim]
        pair = mb[lo:hi, None] * mb[None, lo:hi]
        # pre-scaled by sqrt(DH) (exp applies scale=DH**-0.5 to S + bias);
        # masked entries use -4000 (fp16-safe; * SCALE -> -354, exp -> 0)
        blk = blk * (DH ** 0.5) + np.where(pair > 0, 0.0, -4000.0)[None]
        comb[b] = blk

    def headmajor(w):
        # [128, e*H + 128*h + m] <- W[128h+p, 128e+m]
        a = np.asarray(w, np.float32).reshape(NH_T, 128, HEADS, DH)
        return np.ascontiguousarray(
            a.transpose(1, 2, 0, 3).reshape(128, HEADS, H))

    wqh = headmajor(Wq)
    wkh = headmajor(Wk)
    wv3 = np.asarray(Wv, np.float32).reshape(H, HEADS, DH)
    wo3 = np.asarray(Wo, np.float32).reshape(HEADS, DH, H)
    ident = np.eye(128, dtype=np.float16)

    in_maps = []
    for c in range(NCORES):
        # program-head order: real heads 2c, 2c+1 first (they own the shared
        # block 40 on this core), then the rest in order
        perm = [2 * c, 2 * c + 1] + [e for e in range(HEADS)
                                     if e not in (2 * c, 2 * c + 1)]
        xc = np.zeros((TPAD, H), np.float32)
        for j in range(NB_CORE):
            b = NB_OWN * c + j if j < NB_OWN else NB - 1
            xc[BPAD * j:BPAD * j + KBLK] = xp[KBLK * b:KBLK * (b + 1)]
        # bias layout: rows 0-49 / 64-113 = even/odd block of each pair,
        # cols (blockpair, headgroup, head, k); injected by one ident matmul.
        # Block 5 (= real block 40) only has program heads 0-1; its other
        # slots stay zero.
        bc = np.zeros((128, 3 * HEADS * KBLK), np.float16)
        for b in range(NB_CORE):
            if b < NB_OWN:
                blkbias = comb[NB_OWN * c + b][perm]      # [16, 50, 50]
            else:
                blkbias = np.zeros((HEADS, KBLK, KBLK), np.float32)
                blkbias[0:2] = comb[NB - 1][[2 * c, 2 * c + 1]]
            bp, par = b // 2, b % 2
            for g in range(2):
                blk = blkbias[8 * g:8 * (g + 1)]          # [8, 50, 50]
                bc[BPAD * par:BPAD * par + KBLK,
                   (bp * 2 + g) * 8 * KBLK:(bp * 2 + g + 1) * 8 * KBLK] = (
                    blk.transpose(1, 0, 2).reshape(KBLK, 8 * KBLK))
        # xT layout: [128, h*TPAD + t] <- x[t, 128h+p]
        xh = np.ascontiguousarray(
            xc.T.reshape(NH_T, 128, TPAD).transpose(1, 0, 2).reshape(
                128, NH_T * TPAD)).astype(np.float16)
        in_maps.append({
            "xT": xh,
            "wq": np.ascontiguousarray(
                wqh[:, perm].reshape(128, HEADS * H)).astype(np.float16),
            "wk": np.ascontiguousarray(
                wkh[:, perm].reshape(128, HEADS * H)).astype(np.float16),
            "wv": np.ascontiguousarray(
                wv3[:, perm].reshape(H, H)).astype(np.float16),
            "wo": np.ascontiguousarray(
                wo3[perm].reshape(H, H)).astype(np.float16),
            "bias": np.ascontiguousarray(bc),
            "ident": ident,
        })
    return in_maps


def _gather(results):
    out = np.empty((T, H), np.float32)
    y40 = np.zeros((KBLK, H), np.float32)
    for c in range(NCORES):
        yc = np.asarray(results[c]["y"], np.float32).T   # [TPACK, H]
        for j in range(NB_OWN):
            b = NB_OWN * c + j
            out[KBLK * b:KBLK * (b + 1)] = yc[KBLK * j:KBLK * (j + 1)]
        # cols 250-300 hold this core's 2-head partial of shared block 40
        y40 += yc[TFULL:TPACK]
    n40 = T - KBLK * (NB - 1)                            # 48 real rows
    out[KBLK * (NB - 1):] = y40[:n40]
    return out


def run(trace=False, **inputs):
    if "nc" not in _CACHE:
        _CACHE["nc"] = _build_program()
    nc = _CACHE["nc"]
    in_maps = _prep_inputs(
        inputs["x_BxTxH"], inputs["mask_BxT"], inputs["attention_bias_BxHxTxT"],
        inputs["Wq"], inputs["Wk"], inputs["Wv"], inputs["Wo"])
    res = run_bass_kernel_spmd(nc, in_maps, list(range(NCORES)), trace=trace)
    out = _gather(res.results)[None]       # restore batch dim [1, T, H]
    return out, res.exec_time_ns


def kernel(**inputs):
    out, _ = run(trace=False, **inputs)
    return out

